# revision 9
# baseline (speedup 1.0000x reference)
"""Trainium2 Bass kernel for nn_JacobiMachine: 100 masked-Jacobi steps on a
1024x1024 grid.

The reference's multigrid machinery is dead code (its prolongation loop writes
grids[level+1], never grids[0]), so the output is exactly
jacobi(X, Mask1, 100) with x <- conv(x)*m + x*(1-m), conv = 0.25*(N+S+E+W),
zero boundary.

Distribution: 1D row sharding, 128 rows per core, no runtime communication.
Each core redundantly evolves a 384-row region (3 tiles of [128, 1024])
covering its +/-100-row influence cone. Validity shrinks 1 row/iter from the
region's outer edges; the 28-row slack keeps the owned tile exact.

Formulation: residual r = N+S+E+W-4x kept in PSUM (3 tiles = 6 banks).
Per iteration: u = 0.25*m * r (DVE), r += S(u) via PE matmuls
(Sv = tridiag(1,-4,1) vertical, shifted-identity horizontal, rank-1
inter-tile couplings), and sum_u accumulated in PSUM (2 banks) for the owned
tile. Output = x_own + sum_u. Zero BC and out-of-grid rows are handled by
zero-padding the mask (forces u=0 there). All 8 cores run an identical
program; only their input data differs.
"""

import numpy as np
from contextlib import ExitStack

NT = 100
N = 1024
P = 128
NCORES = 8

_CACHE = {}


def _build_program():
    import concourse.bacc as bacc
    import concourse.mybir as mybir
    from concourse.tile import TileContext

    dt = mybir.dt.float32
    nc = bacc.Bacc("TRN2", target_bir_lowering=False, debug=False,
                   num_devices=NCORES)

    x3_d = nc.dram_tensor("x3", (3 * P, N), dt, kind="ExternalInput")
    m43_d = nc.dram_tensor("m43", (3 * P, N), dt, kind="ExternalInput")
    sv_d = nc.dram_tensor("sv", (P, P), dt, kind="ExternalInput")
    id_d = nc.dram_tensor("ident", (P, P), dt, kind="ExternalInput")
    e0_d = nc.dram_tensor("e0", (P, P), dt, kind="ExternalInput")
    e127_d = nc.dram_tensor("e127", (P, P), dt, kind="ExternalInput")
    out_d = nc.dram_tensor("out", (P, N), dt, kind="ExternalOutput")

    with TileContext(nc) as tc:
        with ExitStack() as ctx:
            cpool = ctx.enter_context(tc.tile_pool(name="const", bufs=1))
            dpool = ctx.enter_context(tc.tile_pool(name="data", bufs=1))
            upool = ctx.enter_context(tc.tile_pool(name="u", bufs=2))
            ppool = ctx.enter_context(
                tc.tile_pool(name="psum", bufs=1, space="PSUM"))

            svT = cpool.tile([P, P], dt, tag="sv")
            nc.sync.dma_start(out=svT[:], in_=sv_d.ap()[:])
            idT = cpool.tile([P, P], dt, tag="ident")
            nc.sync.dma_start(out=idT[:], in_=id_d.ap()[:])
            e0T = cpool.tile([P, P], dt, tag="e0")
            nc.sync.dma_start(out=e0T[:], in_=e0_d.ap()[:])
            e127T = cpool.tile([P, P], dt, tag="e127")
            nc.sync.dma_start(out=e127T[:], in_=e127_d.ap()[:])

            xt, m4t = [], []
            for t in range(3):
                xtile = dpool.tile([P, N], dt, tag=f"x{t}")
                nc.sync.dma_start(out=xtile[:],
                                  in_=x3_d.ap()[P * t:P * (t + 1), :])
                xt.append(xtile)
                mtile = dpool.tile([P, N], dt, tag=f"m4{t}")
                nc.sync.dma_start(out=mtile[:],
                                  in_=m43_d.ap()[P * t:P * (t + 1), :])
                m4t.append(mtile)

            r = [ppool.tile([P, N], dt, tag=f"r{t}", name=f"r{t}")
                 for t in range(3)]
            su = ppool.tile([P, N], dt, tag="su")

            H = N // 2  # 512

            def stencil(dst, src, first, last=False):
                # dst += Sv @ src  (vertical within tile, diag -4)
                for j in (0, H):
                    nc.tensor.matmul(dst[:, j:j + H], svT[:],
                                     src[:, j:j + H], start=first, stop=False,
                                     skip_group_check=True)
                # dst[:, 1:] += src[:, :-1]  (west neighbor)
                nc.tensor.matmul(dst[:, 1:H], idT[:],
                                 src[:, 0:H - 1], start=False, stop=False,
                                 skip_group_check=True)
                nc.tensor.matmul(dst[:, H:N], idT[:],
                                 src[:, H - 1:N - 1], start=False, stop=False,
                                 skip_group_check=True)
                # dst[:, :-1] += src[:, 1:]  (east neighbor)
                nc.tensor.matmul(dst[:, 0:H], idT[:],
                                 src[:, 1:H + 1], start=False, stop=False,
                                 skip_group_check=True)
                nc.tensor.matmul(dst[:, H:N - 1], idT[:],
                                 src[:, H + 1:N], start=False, stop=last,
                                     skip_group_check=True)

            def couple(src3, last=False):
                # rank-1 inter-tile vertical couplings
                for j in (0, H):
                    # r1 row 0 += src0 row 127
                    nc.tensor.matmul(r[1][:, j:j + H], e0T[:],
                                     src3[0][:, j:j + H],
                                     start=False, stop=False,
                                 skip_group_check=True)
                    # r0 row 127 += src1 row 0
                    nc.tensor.matmul(r[0][:, j:j + H], e127T[:],
                                     src3[1][:, j:j + H],
                                     start=False, stop=last,
                                     skip_group_check=True)
                    # r1 row 127 += src2 row 0
                    nc.tensor.matmul(r[1][:, j:j + H], e127T[:],
                                     src3[2][:, j:j + H],
                                     start=False, stop=last,
                                     skip_group_check=True)
                    # r2 row 0 += src1 row 127
                    nc.tensor.matmul(r[2][:, j:j + H], e0T[:],
                                     src3[1][:, j:j + H],
                                     start=False, stop=last,
                                     skip_group_check=True)

            # r = S(x)
            for t in range(3):
                stencil(r[t], xt[t], first=True)
            couple(xt)

            for k in range(NT):
                u3 = []
                for t in range(3):
                    u = upool.tile([P, N], dt, tag=f"u{t}")
                    nc.vector.tensor_mul(out=u[:], in0=m4t[t][:], in1=r[t][:])
                    u3.append(u)
                # sum_u += u1
                for j in (0, H):
                    nc.tensor.matmul(su[:, j:j + H], idT[:],
                                     u3[1][:, j:j + H],
                                     start=(k == 0), stop=(k == NT - 1),
                                     skip_group_check=True)
                if k < NT - 1:
                    last = k == NT - 2
                    for t in range(3):
                        stencil(r[t], u3[t], first=False, last=last)
                    couple(u3, last=last)

            out_t = dpool.tile([P, N], dt, tag="out")
            nc.vector.tensor_add(out=out_t[:], in0=xt[1][:], in1=su[:])
            nc.sync.dma_start(out=out_d.ap()[:], in_=out_t[:])

    nc.compile()
    return nc


def _get_program():
    if "nc" not in _CACHE:
        _CACHE["nc"] = _build_program()
    return _CACHE["nc"]


def _make_weights():
    sv = np.zeros((P, P), dtype=np.float32)
    for i in range(P):
        sv[i, i] = -4.0
        if i > 0:
            sv[i, i - 1] = 1.0
        if i < P - 1:
            sv[i, i + 1] = 1.0
    ident = np.eye(P, dtype=np.float32)
    e0 = np.zeros((P, P), dtype=np.float32)
    e0[P - 1, 0] = 1.0   # out row 0 <- src row 127
    e127 = np.zeros((P, P), dtype=np.float32)
    e127[0, P - 1] = 1.0  # out row 127 <- src row 0
    return sv, ident, e0, e127


def _prepare_in_maps(inputs):
    X = np.ascontiguousarray(np.asarray(inputs["X"], dtype=np.float32)[0, 0])
    M = np.ascontiguousarray(
        np.asarray(inputs["Mask1"], dtype=np.float32)[0, 0])

    x_pad = np.zeros((N + 2 * P, N), dtype=np.float32)
    x_pad[P:P + N] = X
    m4_pad = np.zeros((N + 2 * P, N), dtype=np.float32)
    m4_pad[P:P + N] = 0.25 * M

    sv, ident, e0, e127 = _make_weights()
    in_maps = []
    for c in range(NCORES):
        lo = P * c
        in_maps.append({
            "x3": np.ascontiguousarray(x_pad[lo:lo + 3 * P]),
            "m43": np.ascontiguousarray(m4_pad[lo:lo + 3 * P]),
            "sv": sv, "ident": ident, "e0": e0, "e127": e127,
        })
    return in_maps


def kernel(**inputs) -> np.ndarray:
    from concourse.bass_utils import run_bass_kernel_spmd

    in_maps = _prepare_in_maps(inputs)
    nc = _get_program()
    res = run_bass_kernel_spmd(nc, in_maps, core_ids=list(range(NCORES)))
    out = np.zeros((N, N), dtype=np.float32)
    for c in range(NCORES):
        out[P * c:P * (c + 1)] = res.results[c]["out"]
    return out[None, None]


def _install_ntff_hook():
    """antenv.axon_hooks is absent on this image; synthesize it so
    run_bass_kernel_spmd(trace=True) can NTFF-profile via libaxon_pjrt."""
    import sys, types
    if "antenv.axon_hooks" in sys.modules:
        return
    import antenv
    from trn_agent_boot import trn_boot
    mod = types.ModuleType("antenv.axon_hooks")
    _state = {"hook": trn_boot._ntff_profile_via_ctypes(
        "/opt/axon/libaxon_pjrt.so")}
    mod.get_axon_ntff_profile_hook = lambda: _state["hook"]
    mod.set_axon_ntff_profile_hook = lambda h: _state.__setitem__("hook", h)
    sys.modules["antenv.axon_hooks"] = mod
    antenv.axon_hooks = mod


def timed_run(**inputs):
    """Profiled run; returns HW exec time in ns (slowest traced core)."""
    from concourse.bass_utils import run_bass_kernel_spmd

    _install_ntff_hook()
    in_maps = _prepare_in_maps(inputs)
    nc = _get_program()
    res = run_bass_kernel_spmd(nc, in_maps, core_ids=list(range(NCORES)),
                               trace=True)
    return res.exec_time_ns


# revision 10
# speedup vs baseline: 3.2250x; 3.2250x over previous
"""Trainium2 Bass kernel for nn_JacobiMachine: 100 masked-Jacobi steps on a
1024x1024 grid.

The reference's multigrid machinery is dead code (its prolongation loop writes
grids[level+1], never grids[0]), so the output is exactly
jacobi(X, Mask1, 100) with x <- conv(x)*m + x*(1-m), conv = 0.25*(N+S+E+W),
zero boundary.

Distribution: 1D row sharding, 128 rows per core, no runtime communication.
Each core redundantly evolves a 384-row region (3 tiles of [128, 1024])
covering its +/-100-row influence cone. Validity shrinks 1 row/iter from the
region's outer edges; the 28-row slack keeps the owned tile exact.

Formulation: residual r = N+S+E+W-4x kept in PSUM (3 tiles = 6 banks).
Per iteration: u = 0.25*m * r (DVE), r += S(u) via PE matmuls
(Sv = tridiag(1,-4,1) vertical, shifted-identity horizontal, rank-1
inter-tile couplings), and sum_u accumulated in PSUM (2 banks) for the owned
tile. Output = x_own + sum_u. Zero BC and out-of-grid rows are handled by
zero-padding the mask (forces u=0 there). All 8 cores run an identical
program; only their input data differs.
"""

import numpy as np
from contextlib import ExitStack

NT = 100
N = 1024
P = 128
NCORES = 8

_CACHE = {}


def _build_program():
    import concourse.bacc as bacc
    import concourse.mybir as mybir
    from concourse.tile import TileContext

    dt = mybir.dt.float32
    bt = mybir.dt.bfloat16
    nc = bacc.Bacc("TRN2", target_bir_lowering=False, debug=False,
                   num_devices=NCORES)

    x3_d = nc.dram_tensor("x3", (3 * P, N), dt, kind="ExternalInput")
    m43_d = nc.dram_tensor("m43", (3 * P, N), dt, kind="ExternalInput")
    sv_d = nc.dram_tensor("sv", (P, P), dt, kind="ExternalInput")
    id_d = nc.dram_tensor("ident", (P, P), dt, kind="ExternalInput")
    e0_d = nc.dram_tensor("e0", (P, P), dt, kind="ExternalInput")
    e127_d = nc.dram_tensor("e127", (P, P), dt, kind="ExternalInput")
    svb_d = nc.dram_tensor("svb", (P, P), bt, kind="ExternalInput")
    idb_d = nc.dram_tensor("identb", (P, P), bt, kind="ExternalInput")
    e0b_d = nc.dram_tensor("e0b", (P, P), bt, kind="ExternalInput")
    e127b_d = nc.dram_tensor("e127b", (P, P), bt, kind="ExternalInput")
    out_d = nc.dram_tensor("out", (P, N), dt, kind="ExternalOutput")

    with TileContext(nc) as tc:
        with ExitStack() as ctx:
            cpool = ctx.enter_context(tc.tile_pool(name="const", bufs=1))
            dpool = ctx.enter_context(tc.tile_pool(name="data", bufs=1))
            upool = ctx.enter_context(tc.tile_pool(name="u", bufs=2))
            ppool = ctx.enter_context(
                tc.tile_pool(name="psum", bufs=1, space="PSUM"))

            svT = cpool.tile([P, P], dt, tag="sv")
            nc.sync.dma_start(out=svT[:], in_=sv_d.ap()[:])
            idT = cpool.tile([P, P], dt, tag="ident")
            nc.sync.dma_start(out=idT[:], in_=id_d.ap()[:])
            e0T = cpool.tile([P, P], dt, tag="e0")
            nc.sync.dma_start(out=e0T[:], in_=e0_d.ap()[:])
            e127T = cpool.tile([P, P], dt, tag="e127")
            nc.sync.dma_start(out=e127T[:], in_=e127_d.ap()[:])
            svB = cpool.tile([P, P], bt, tag="svb")
            nc.sync.dma_start(out=svB[:], in_=svb_d.ap()[:])
            idB = cpool.tile([P, P], bt, tag="identb")
            nc.sync.dma_start(out=idB[:], in_=idb_d.ap()[:])
            e0B = cpool.tile([P, P], bt, tag="e0b")
            nc.sync.dma_start(out=e0B[:], in_=e0b_d.ap()[:])
            e127B = cpool.tile([P, P], bt, tag="e127b")
            nc.sync.dma_start(out=e127B[:], in_=e127b_d.ap()[:])

            xt, m4t = [], []
            for t in range(3):
                xtile = dpool.tile([P, N], dt, tag=f"x{t}")
                nc.sync.dma_start(out=xtile[:],
                                  in_=x3_d.ap()[P * t:P * (t + 1), :])
                xt.append(xtile)
                mtile = dpool.tile([P, N], dt, tag=f"m4{t}")
                nc.sync.dma_start(out=mtile[:],
                                  in_=m43_d.ap()[P * t:P * (t + 1), :])
                m4t.append(mtile)

            r = [ppool.tile([P, N], dt, tag=f"r{t}", name=f"r{t}")
                 for t in range(3)]
            su = ppool.tile([P, N], dt, tag="su")

            H = N // 2  # 512

            def stencil(dst, src, first, last=False, W=None):
                wSv, wId = W
                # dst += Sv @ src  (vertical within tile, diag -4)
                for j in (0, H):
                    nc.tensor.matmul(dst[:, j:j + H], wSv[:],
                                     src[:, j:j + H], start=first, stop=False,
                                     skip_group_check=True)
                # dst[:, 1:] += src[:, :-1]  (west neighbor)
                nc.tensor.matmul(dst[:, 1:H], wId[:],
                                 src[:, 0:H - 1], start=False, stop=False,
                                 skip_group_check=True)
                nc.tensor.matmul(dst[:, H:N], wId[:],
                                 src[:, H - 1:N - 1], start=False, stop=False,
                                 skip_group_check=True)
                # dst[:, :-1] += src[:, 1:]  (east neighbor)
                nc.tensor.matmul(dst[:, 0:H], wId[:],
                                 src[:, 1:H + 1], start=False, stop=False,
                                 skip_group_check=True)
                nc.tensor.matmul(dst[:, H:N - 1], wId[:],
                                 src[:, H + 1:N], start=False, stop=last,
                                     skip_group_check=True)

            def couple(src3, last=False, W=None):
                wE0, wE127 = W
                # rank-1 inter-tile vertical couplings
                for j in (0, H):
                    # r1 row 0 += src0 row 127
                    nc.tensor.matmul(r[1][:, j:j + H], wE0[:],
                                     src3[0][:, j:j + H],
                                     start=False, stop=False,
                                 skip_group_check=True)
                    # r0 row 127 += src1 row 0
                    nc.tensor.matmul(r[0][:, j:j + H], wE127[:],
                                     src3[1][:, j:j + H],
                                     start=False, stop=last,
                                     skip_group_check=True)
                    # r1 row 127 += src2 row 0
                    nc.tensor.matmul(r[1][:, j:j + H], wE127[:],
                                     src3[2][:, j:j + H],
                                     start=False, stop=last,
                                     skip_group_check=True)
                    # r2 row 0 += src1 row 127
                    nc.tensor.matmul(r[2][:, j:j + H], wE0[:],
                                     src3[1][:, j:j + H],
                                     start=False, stop=last,
                                     skip_group_check=True)

            # r = S(x)  (fp32 weights, fp32 x)
            for t in range(3):
                stencil(r[t], xt[t], first=True, W=(svT, idT))
            couple(xt, W=(e0T, e127T))

            for k in range(NT):
                u3 = []
                for t in range(3):
                    u = upool.tile([P, N], bt, tag=f"u{t}")
                    nc.vector.tensor_mul(out=u[:], in0=m4t[t][:], in1=r[t][:])
                    u3.append(u)
                # sum_u += u1
                for j in (0, H):
                    nc.tensor.matmul(su[:, j:j + H], idB[:],
                                     u3[1][:, j:j + H],
                                     start=(k == 0), stop=(k == NT - 1),
                                     skip_group_check=True)
                if k < NT - 1:
                    last = k == NT - 2
                    for t in range(3):
                        stencil(r[t], u3[t], first=False, last=last,
                                W=(svB, idB))
                    couple(u3, last=last, W=(e0B, e127B))

            out_t = dpool.tile([P, N], dt, tag="out")
            nc.vector.tensor_add(out=out_t[:], in0=xt[1][:], in1=su[:])
            nc.sync.dma_start(out=out_d.ap()[:], in_=out_t[:])

    nc.compile()
    return nc


def _get_program():
    if "nc" not in _CACHE:
        _CACHE["nc"] = _build_program()
    return _CACHE["nc"]


def _make_weights():
    sv = np.zeros((P, P), dtype=np.float32)
    for i in range(P):
        sv[i, i] = -4.0
        if i > 0:
            sv[i, i - 1] = 1.0
        if i < P - 1:
            sv[i, i + 1] = 1.0
    ident = np.eye(P, dtype=np.float32)
    e0 = np.zeros((P, P), dtype=np.float32)
    e0[P - 1, 0] = 1.0   # out row 0 <- src row 127
    e127 = np.zeros((P, P), dtype=np.float32)
    e127[0, P - 1] = 1.0  # out row 127 <- src row 0
    return sv, ident, e0, e127


def _prepare_in_maps(inputs):
    X = np.ascontiguousarray(np.asarray(inputs["X"], dtype=np.float32)[0, 0])
    M = np.ascontiguousarray(
        np.asarray(inputs["Mask1"], dtype=np.float32)[0, 0])

    x_pad = np.zeros((N + 2 * P, N), dtype=np.float32)
    x_pad[P:P + N] = X
    m4_pad = np.zeros((N + 2 * P, N), dtype=np.float32)
    m4_pad[P:P + N] = 0.25 * M

    sv, ident, e0, e127 = _make_weights()
    import ml_dtypes
    bf = ml_dtypes.bfloat16
    in_maps = []
    for c in range(NCORES):
        lo = P * c
        in_maps.append({
            "x3": np.ascontiguousarray(x_pad[lo:lo + 3 * P]),
            "m43": np.ascontiguousarray(m4_pad[lo:lo + 3 * P]),
            "sv": sv, "ident": ident, "e0": e0, "e127": e127,
            "svb": sv.astype(bf), "identb": ident.astype(bf),
            "e0b": e0.astype(bf), "e127b": e127.astype(bf),
        })
    return in_maps


def kernel(**inputs) -> np.ndarray:
    from concourse.bass_utils import run_bass_kernel_spmd

    in_maps = _prepare_in_maps(inputs)
    nc = _get_program()
    res = run_bass_kernel_spmd(nc, in_maps, core_ids=list(range(NCORES)))
    out = np.zeros((N, N), dtype=np.float32)
    for c in range(NCORES):
        out[P * c:P * (c + 1)] = res.results[c]["out"]
    return out[None, None]


def _install_ntff_hook():
    """antenv.axon_hooks is absent on this image; synthesize it so
    run_bass_kernel_spmd(trace=True) can NTFF-profile via libaxon_pjrt."""
    import sys, types
    if "antenv.axon_hooks" in sys.modules:
        return
    import antenv
    from trn_agent_boot import trn_boot
    mod = types.ModuleType("antenv.axon_hooks")
    _state = {"hook": trn_boot._ntff_profile_via_ctypes(
        "/opt/axon/libaxon_pjrt.so")}
    mod.get_axon_ntff_profile_hook = lambda: _state["hook"]
    mod.set_axon_ntff_profile_hook = lambda h: _state.__setitem__("hook", h)
    sys.modules["antenv.axon_hooks"] = mod
    antenv.axon_hooks = mod


def timed_run(**inputs):
    """Profiled run; returns HW exec time in ns (slowest traced core)."""
    from concourse.bass_utils import run_bass_kernel_spmd

    _install_ntff_hook()
    in_maps = _prepare_in_maps(inputs)
    nc = _get_program()
    res = run_bass_kernel_spmd(nc, in_maps, core_ids=list(range(NCORES)),
                               trace=True)
    return res.exec_time_ns
